# revision 3
# baseline (speedup 1.0000x reference)
"""Trainium2 Bass kernel for DynamicMaskHead (CondInst-style dynamic conv mask head).

Strategy
--------
256 instances sharded 32/core across 8 NeuronCores (2 "packs" of 16 per core).

Conv stage (3 x 1x1 conv per instance == tiny per-instance GEMMs over HW=24576):
  * The rel-coord channels are folded through the per-instance weights on the
    host:  w0 . rel = w0 . (loc - p)/soi  ==  (folded coeffs) . [w_grid, h_grid]
    + folded bias.  The integer w/h grids are bf16-exact, so every instance of
    a pack shares ONE moving operand
        xt = [w_grid, h_grid, feats_im0 (8), feats_im1 (8), ones]  (19 x HW)
    (each instance's stationary column block holds its feats weights in the
    row-block of ITS image and zeros in the other; the ones-row carries b0).
  * Layer0: stationary [19,128] per pack (16 inst x 8 ch), one matmul per
    spatial chunk.  Layer1: 128x128 block-diagonal stationary.  Layer2:
    [128,16] stationary; pack A lands in PSUM rows 0:16, pack B in rows 32:48
    (tile_position col groups 0 / 1).  b2 is added on the HOST at the end
    (bilinear upsample of a constant is the constant).

Upsample (x2 bilinear, align_corners) as two matmuls per instance:
  out^T = C^T . (v^T . R^T)   with v = z_i reshaped [H=128, W=192]
  * tmpT = v^T R^T : stationary = v tiles, moving = constant R^T [128,256]
  * outT = C^T tmpT: stationary = constant C chunks, moving = tmpT
  Device emits out^T [384,256] per instance; the host transposes for free.

All intermediates bf16 (inputs exact where it matters), PSUM accumulation fp32,
final output fp32.
"""

import sys

for _p in ("/opt/trn_rl_repo",):
    if _p not in sys.path:
        sys.path.insert(0, _p)

import numpy as np
import ml_dtypes

import concourse.bass as bass
import concourse.bacc as bacc
import concourse.mybir as mybir
import concourse.tile as tile
from concourse.bass_utils import run_bass_kernel_spmd

BF16 = ml_dtypes.bfloat16
CH, CIN = 8, 8
N_IMG, H, W = 2, 128, 192
HW = H * W
N_INST, N_CORES, PER_CORE, PACK = 256, 8, 32, 16
H2, W2 = 2 * H, 2 * W  # 256, 384
CHUNK = 512
NCHUNK = HW // CHUNK  # 48

_CACHE = {}


def _interp_mats():
    """R [H2,H] (row interp), C [W,W2] (col interp), matching the reference's
    align_corners=True fp32 arithmetic."""
    ys = np.arange(H2, dtype=np.float32) * np.float32((H - 1) / (H2 - 1))
    y0 = np.floor(ys).astype(np.int32)
    y1 = np.minimum(y0 + 1, H - 1)
    wy = (ys - y0.astype(np.float32)).astype(np.float32)
    R = np.zeros((H2, H), np.float32)
    R[np.arange(H2), y0] += 1.0 - wy
    R[np.arange(H2), y1] += wy

    xs = np.arange(W2, dtype=np.float32) * np.float32((W - 1) / (W2 - 1))
    x0 = np.floor(xs).astype(np.int32)
    x1 = np.minimum(x0 + 1, W - 1)
    wx = (xs - x0.astype(np.float32)).astype(np.float32)
    C = np.zeros((W, W2), np.float32)
    C[x0, np.arange(W2)] += 1.0 - wx
    C[x1, np.arange(W2)] += wx
    return R, C


def _build_program():
    nc = bacc.Bacc("TRN2", target_bir_lowering=False, debug=False)
    dt = mybir.dt
    BF, FP = dt.bfloat16, dt.float32

    xt_d = nc.dram_tensor("xt", [19, HW], BF, kind="ExternalInput").ap()
    w0_d = nc.dram_tensor("w0", [19, 2, 128], BF, kind="ExternalInput").ap()
    w1_d = nc.dram_tensor("w1", [128, 2, 128], BF, kind="ExternalInput").ap()
    w2_d = nc.dram_tensor("w2", [128, 2, 32], BF, kind="ExternalInput").ap()
    b1_d = nc.dram_tensor("b1", [128, 2, 1], FP, kind="ExternalInput").ap()
    rt_d = nc.dram_tensor("rt", [H, H2], BF, kind="ExternalInput").ap()
    c_d = nc.dram_tensor("cmat", [W, W2], BF, kind="ExternalInput").ap()
    out_d = nc.dram_tensor("out", [PER_CORE, W2, H2], FP, kind="ExternalOutput").ap()

    with tile.TileContext(nc) as tc:
        _kernel_body(tc, xt_d, w0_d, w1_d, w2_d, b1_d, rt_d, c_d, out_d)
    nc.compile()
    return nc


def _kernel_body(tc, xt_d, w0_d, w1_d, w2_d, b1_d, rt_d, c_d, out_d):
    nc = tc.nc
    dt = mybir.dt
    BF, FP = dt.bfloat16, dt.float32
    Relu = mybir.ActivationFunctionType.Relu
    Copy = mybir.ActivationFunctionType.Copy

    with (
        tc.tile_pool(name="const", bufs=1) as cpool,
        tc.tile_pool(name="big", bufs=1) as bigpool,
    ):
        xt = cpool.tile([19, HW], BF)
        nc.sync.dma_start(xt[:], xt_d[:])
        w0 = cpool.tile([19, 2, 128], BF)
        nc.sync.dma_start(w0[:], w0_d[:])
        w1 = cpool.tile([128, 2, 128], BF)
        nc.sync.dma_start(w1[:], w1_d[:])
        w2 = cpool.tile([128, 2, 32], BF)
        nc.sync.dma_start(w2[:], w2_d[:])
        b1 = cpool.tile([128, 2, 1], FP)
        nc.sync.dma_start(b1[:], b1_d[:])
        rt = cpool.tile([H, H2], BF)
        nc.sync.dma_start(rt[:], rt_d[:])
        c0 = cpool.tile([128, W2], BF)
        nc.sync.dma_start(c0[:], c_d[0:128, :])
        c1 = cpool.tile([W - 128, W2], BF)
        nc.sync.dma_start(c1[:], c_d[128:W, :])

        zsb = bigpool.tile([64, HW], BF)  # rows 0:16 pack A, 32:48 pack B
        vsb = bigpool.tile([128, PER_CORE * W], BF)  # per-inst [H,W] views

        # ---------------- conv stage ----------------
        with (
            tc.tile_pool(name="ps0", bufs=2, space="PSUM") as ps0p,
            tc.tile_pool(name="ps1", bufs=2, space="PSUM") as ps1p,
            tc.tile_pool(name="zps", bufs=2, space="PSUM") as zpsp,
            tc.tile_pool(name="work", bufs=3) as wpool,
        ):
            for t in range(NCHUNK):
                sl = bass.ts(t, CHUNK)
                zps = zpsp.tile([64, CHUNK], FP)
                for p in range(2):
                    ps0 = ps0p.tile([128, CHUNK], FP)
                    nc.tensor.matmul(
                        ps0[:], w0[:, p, :], xt[:, sl], start=True, stop=True
                    )
                    y0 = wpool.tile([128, CHUNK], BF, tag="y0")
                    nc.vector.tensor_scalar_max(y0[:], ps0[:], 0.0)
                    ps1 = ps1p.tile([128, CHUNK], FP)
                    nc.tensor.matmul(ps1[:], w1[:, p, :], y0[:], start=True, stop=True)
                    y1 = wpool.tile([128, CHUNK], BF, tag="y1")
                    nc.scalar.activation(y1[:], ps1[:], Relu, bias=b1[:, p, :])
                    nc.tensor.matmul(
                        zps[32 * p : 32 * p + 32, :],
                        w2[:, p, :],
                        y1[:],
                        start=True,
                        stop=True,
                        tile_position=(0, 32 * p),
                    )
                nc.vector.tensor_copy(zsb[:, sl], zps[:])

        # ---------------- scatter z -> per-instance [H, W] tiles ----------------
        for i in range(PER_CORE):
            zrow = i if i < PACK else i + PACK
            nc.sync.dma_start(
                vsb[:, i * W : (i + 1) * W],
                zsb[zrow : zrow + 1, :].rearrange("o (h w) -> o h w", h=H),
            )

        # ---------------- upsample stage ----------------
        with (
            tc.tile_pool(name="tps", bufs=2, space="PSUM") as tpsp,
            tc.tile_pool(name="ops", bufs=2, space="PSUM") as opsp,
            tc.tile_pool(name="ut", bufs=3) as upool,
        ):
            for i in range(PER_CORE):
                v = vsb[:, i * W : (i + 1) * W]
                tps0 = tpsp.tile([128, H2], FP, tag="tps0")
                nc.tensor.matmul(tps0[:], v[:, 0:128], rt[:], start=True, stop=True)
                tps1 = tpsp.tile([W - 128, H2], FP, tag="tps1")
                nc.tensor.matmul(tps1[:], v[:, 128:W], rt[:], start=True, stop=True)
                t0 = upool.tile([128, H2], BF, tag="t0")
                nc.scalar.activation(t0[:], tps0[:], Copy)
                t1 = upool.tile([W - 128, H2], BF, tag="t1")
                nc.scalar.activation(t1[:], tps1[:], Copy)
                osb = upool.tile([128, 3 * H2], FP, tag="osb")
                for m in range(3):
                    ops = opsp.tile([128, H2], FP)
                    nc.tensor.matmul(
                        ops[:],
                        c0[:, m * 128 : (m + 1) * 128],
                        t0[:],
                        start=True,
                        stop=False,
                    )
                    nc.tensor.matmul(
                        ops[:],
                        c1[:, m * 128 : (m + 1) * 128],
                        t1[:],
                        start=False,
                        stop=True,
                    )
                    nc.vector.tensor_copy(osb[:, m * H2 : (m + 1) * H2], ops[:])
                for m in range(3):
                    nc.sync.dma_start(
                        out_d[i, m * 128 : (m + 1) * 128, :],
                        osb[:, m * H2 : (m + 1) * H2],
                    )


def _host_pack(params, locs, soi, im, stride):
    """Per-core host-side weight packing. params [32,169] etc (fp32 numpy)."""
    n = params.shape[0]
    w0 = params[:, 0:80].reshape(n, CH, CIN + 2)
    w1 = params[:, 80:144].reshape(n, CH, CH)
    w2 = params[:, 144:152].reshape(n, 1, CH)
    b0 = params[:, 152:160]
    b1 = params[:, 160:168]
    b2 = params[:, 168:169]
    half = np.float32(stride // 2)

    lhsT0 = np.zeros((19, 2, 128), np.float32)
    lhsT1 = np.zeros((128, 2, 128), np.float32)
    lhsT2 = np.zeros((128, 2, 32), np.float32)
    b1col = np.zeros((128, 2, 1), np.float32)
    b2vec = np.zeros((n,), np.float32)
    for p in range(2):
        for j in range(PACK):
            g = p * PACK + j
            s = np.float32(1.0) / soi[g].astype(np.float32)
            cols = slice(8 * j, 8 * j + 8)
            lhsT0[0, p, cols] = -np.float32(stride) * w0[g, :, 0] * s
            lhsT0[1, p, cols] = -np.float32(stride) * w0[g, :, 1] * s
            fr = 2 + 8 * int(im[g])
            lhsT0[fr : fr + 8, p, cols] = w0[g, :, 2:10].T
            lhsT0[18, p, cols] = b0[g] + (
                w0[g, :, 0] * (locs[g, 0] - half) + w0[g, :, 1] * (locs[g, 1] - half)
            ) * s
            lhsT1[8 * j : 8 * j + 8, p, cols] = w1[g].T
            b1col[8 * j : 8 * j + 8, p, 0] = b1[g]
            lhsT2[8 * j : 8 * j + 8, p, j] = w2[g, 0, :]
            b2vec[g] = b2[g, 0]
    return (
        lhsT0.astype(BF16),
        lhsT1.astype(BF16),
        lhsT2.astype(BF16),
        b1col,
        b2vec,
    )


def _prepare(inputs):
    mask_feats = np.asarray(inputs["mask_feats"], np.float32)
    params = np.asarray(inputs["mask_head_params"], np.float32)
    locs = np.asarray(inputs["instance_locations"], np.float32)
    soi = np.asarray(inputs["sizes_of_interest"], np.float32)
    im = np.asarray(inputs["im_inds"], np.int64)
    stride = int(np.asarray(inputs["mask_feat_stride"]))

    # Shared moving operand: integer grids (bf16-exact), feats, ones.
    xt = np.empty((19, HW), np.float32)
    xt[0, :] = np.tile(np.arange(W, dtype=np.float32), H)
    xt[1, :] = np.repeat(np.arange(H, dtype=np.float32), W)
    xt[2:10, :] = mask_feats[0].reshape(CIN, HW)
    xt[10:18, :] = mask_feats[1].reshape(CIN, HW)
    xt[18, :] = 1.0
    xt = xt.astype(BF16)

    R, C = _interp_mats()
    rt = R.T.astype(BF16).copy()  # [H, H2]
    cm = C.astype(BF16).copy()  # [W, W2]

    in_maps = []
    b2_all = np.zeros((N_INST,), np.float32)
    for c in range(N_CORES):
        s = slice(c * PER_CORE, (c + 1) * PER_CORE)
        lhsT0, lhsT1, lhsT2, b1col, b2vec = _host_pack(
            params[s], locs[s], soi[s], im[s], stride
        )
        b2_all[s] = b2vec
        in_maps.append(
            {
                "xt": xt,
                "w0": lhsT0,
                "w1": lhsT1,
                "w2": lhsT2,
                "b1": b1col,
                "rt": rt,
                "cmat": cm,
            }
        )
    return in_maps, b2_all


def _get_program():
    if "nc" not in _CACHE:
        _CACHE["nc"] = _build_program()
    return _CACHE["nc"]


def kernel(trace=False, **inputs):
    in_maps, b2_all = _prepare(inputs)
    nc = _get_program()
    res = run_bass_kernel_spmd(nc, in_maps, core_ids=list(range(N_CORES)), trace=trace)
    outs = np.stack([res.results[c]["out"] for c in range(N_CORES)])  # [8,32,W2,H2]
    out = outs.reshape(N_INST, W2, H2).transpose(0, 2, 1)  # [256, H2, W2]
    out = out + b2_all[:, None, None]
    full = out[:, None, :, :].astype(np.float32)
    if trace:
        return full, res
    return full
